# revision 37
# baseline (speedup 1.0000x reference)
"""Trainium2 Bass kernel for nn_ATT0: out[b,i,d] = tanh(x[b,i,d] * mean_j x[b,j,d]).

Full input [512, 128, 64] f32; batch dim sharded across 8 NeuronCores
(64 batches/core).

Per-core layout: the I=128 axis is split into S=16 groups of II=8 rows; the
partition dim is p = b*16 + q (b in [0,8), q in [0,16)).  A "subchunk" is 8
batches: each partition holds that batch's contiguous [II, D] block (2 KiB).
A chunk is V consecutive subchunks stacked along the free dim, so chunk sizes
can ramp (small first chunk -> output DMAs start early; small last chunk ->
short tail) while every stage stays ONE instruction per chunk:

  SP  : in-DMA xt[c][p, v, i, d]; later issues the out-DMAs (ACT stays free)
  DVE : partial[c][p, v, d] = sum_i xt[c]   (strided X-axis reduce)
  PE  : sums[c] = sel.T @ partial[c] into PSUM bank c -- the 0/1 selector
        (sel[k,m] = k//16==m//16) folds the 16 partition groups and replicates
        per-batch sums to all partitions, exactly; independent of V
  DVE/POOL: ot[c] = xt[c] * bcast_i(sums[c])
  ACT : tanh(ot[c] * 1/I)  (for POOL chunks: scaled PSUM->SBUF sums copy
        first, since GpSimd cannot read PSUM, and tanh scale=1)

Raw Bass (no Tile), explicit semaphores, one semaphore per DMA (concurrent
DMAs complete out of order).
"""

from contextlib import ExitStack

import numpy as np

import concourse.bass as bass
from concourse import mybir
from concourse.bass_utils import run_bass_kernel_spmd

B, I, D = 512, 128, 64
N_CORES = 8
BPC = B // N_CORES   # 64 batches per core
NB0 = 8              # batches per subchunk
S = 128 // NB0       # 16
II = I // S          # 8
NSUB = BPC // NB0    # 8 subchunks per core

# Chunk sizes in subchunks (must sum to NSUB, each <= 8 for one PSUM bank).
VS = [1] * 8
# Per-chunk engine class:
#   'd': reduce + multiply on DVE
#   'm': reduce on DVE, multiply on GPSIMD (needs a scaled PSUM->SBUF sums
#        copy on ACT, since GpSimd cannot read PSUM)
#   't': reduce as a tree of adds on GPSIMD, multiply on DVE
#   'h': GPSIMD pre-folds the tile in half (one add), DVE reduces the half
CLS = list('dmmdmddd')
COPY_LEAD = 2
# Chunks whose tanh + output DMA are split into two i-halves so the first
# output bytes hit the DMA engines earlier (fills the pipeline-fill bubble).
SPLIT_OUT = frozenset()

_cache = {}


def _build():
    assert sum(VS) == NSUB and all(1 <= v <= 8 for v in VS)
    nchunk = len(VS)
    assert len(CLS) == nchunk and set(CLS) <= {"d", "m", "t", "h"}
    v0s = np.cumsum([0] + list(VS))[:-1]  # first subchunk of each chunk

    f32 = mybir.dt.float32
    nc = bass.Bass()
    x = nc.dram_tensor("x", [BPC, I, D], f32, kind="ExternalInput")
    y = nc.dram_tensor("y", [BPC, I, D], f32, kind="ExternalOutput")

    # [(w b), (q i), d] -> [b, q, w, i, d]: iteration order matches partition
    # order p = b*S + q (so the (b, q) pair merges into ONE uniform-stride AP
    # dim of count 128: b*I*D == (b*S)*II*D), then (subchunk, i, d) in the
    # free dim; the [i, d] run per (partition, subchunk) is 2 KiB contiguous.
    xw = x[:].rearrange("(w b) (q i) d -> b q w i d", b=NB0, q=S)
    yw = y[:].rearrange("(w b) (q i) d -> b q w i d", b=NB0, q=S)

    with ExitStack() as ctx:
        ec = ctx.enter_context
        sel_t = ec(nc.sbuf_tensor("sel_t", [128, 128], f32))
        sel_i = ec(nc.sbuf_tensor("sel_i", [128, 128], mybir.dt.int32))
        xts = [ec(nc.sbuf_tensor(f"xt{c}", [128, v, II, D], f32))
               for c, v in enumerate(VS)]
        ots = [ec(nc.sbuf_tensor(f"ot{c}", [128, v, II, D], f32))
               for c, v in enumerate(VS)]
        partials = [ec(nc.sbuf_tensor(f"pa{c}", [128, v, D], f32))
                    for c, v in enumerate(VS)]
        # PSUM banks recycled modulo 8; matmuls for c >= 8 add a WAR wait
        # on the previous occupant's multiply (its only reader).
        nbank = min(nchunk, 8)
        psums = [ec(nc.psum_tensor(f"sm{k}", [128, max(VS), D], f32))
                 for k in range(nbank)]
        sums = [psums[c % nbank][:, 0:v] for c, v in enumerate(VS)]
        sums_sb = {c: ec(nc.sbuf_tensor(f"ss{c}", [128, VS[c], D], f32))
                   for c in range(nchunk) if CLS[c] == "m"}
        # tree-add temporaries for 't' chunks
        tr4 = {c: ec(nc.sbuf_tensor(f"t4_{c}", [128, VS[c], II // 2, D], f32))
               for c in range(nchunk) if CLS[c] in ("t", "h")}
        tr2 = {c: ec(nc.sbuf_tensor(f"t2_{c}", [128, VS[c], II // 4, D], f32))
               for c in range(nchunk) if CLS[c] == "t"}

        sel_sem = ec(nc.semaphore("sel_sem"))
        in_sems = [ec(nc.semaphore(f"in_sem{c}")) for c in range(nchunk)]
        out_sems = [ec(nc.semaphore(f"out_sem{c}")) for c in range(nchunk)]
        dve_sem = ec(nc.semaphore("dve_sem"))
        pe_sem = ec(nc.semaphore("pe_sem"))
        act_sem = ec(nc.semaphore("act_sem"))
        pool_sem = ec(nc.semaphore("pool_sem"))
        block = ec(nc.Block())

        # --- precomputed queue orders / semaphore tick values -------------
        # DVE queue: reduces for 'd'/'m' chunks in chunk order; each 'd'/'t'
        # multiply fires one slot later so the PE round-trip is hidden.
        dve_ops = []
        pending = []
        for c in range(nchunk):
            if CLS[c] in ("d", "m"):
                dve_ops.append(("reduce", c))
            elif CLS[c] == "h":
                dve_ops.append(("hreduce", c))
            if pending:
                dve_ops.append(("mult", pending.pop(0)))
            if CLS[c] in ("d", "t", "h"):
                pending.append(c)
        dve_ops += [("mult", c) for c in pending]

        reduce_tick = [0] * nchunk
        mult_tick = [0] * nchunk
        for t, (kind, c) in enumerate(dve_ops, start=1):
            (mult_tick if kind == "mult" else reduce_tick)[c] = t

        # POOL queue: per chunk order, tree-adds for 't' chunks and
        # multiplies for 'm' chunks; one shared pool_sem counts every op.
        pool_ops = []
        for c in range(nchunk):
            if CLS[c] == "t":
                pool_ops += [("add4", c), ("add2", c), ("add1", c)]
            elif CLS[c] == "h":
                pool_ops.append(("add4", c))
            elif CLS[c] == "m":
                pool_ops.append(("mult", c))
        tree_tick = {}
        half_tick = {}
        ptick = 0
        for kind, c in pool_ops:
            ptick += 1
            if kind == "add1":
                tree_tick[c] = ptick
            elif kind == "add4" and CLS[c] == "h":
                half_tick[c] = ptick
            elif kind == "mult":
                mult_tick[c] = ptick

        # ACT queue: stage each 'm' chunk's scaled sums copy `lead` chunks
        # ahead of its tanh (the copy is ready right after matmul c); the
        # lead can be overridden per chunk via COPY_LEADS.
        leads = {c: globals().get("COPY_LEADS", {}).get(c, COPY_LEAD)
                 for c in range(nchunk) if CLS[c] == "m"}
        act_ops = []
        for c in range(nchunk):
            for cc, ld in sorted(leads.items()):
                if cc - ld == c and ("copy", cc, 0) not in act_ops:
                    act_ops.append(("copy", cc, 0))
            if c in SPLIT_OUT:
                act_ops += [("tanh", c, 0), ("tanh", c, 1)]
            else:
                act_ops.append(("tanh", c, 0))
        for c in sorted(leads):
            if ("copy", c, 0) not in act_ops:
                act_ops.insert(0, ("copy", c, 0))
        copy_tick = {}
        tanh_tick = {}   # (c, part) -> act tick
        for t, (kind, c, part) in enumerate(act_ops, start=1):
            if kind == "copy":
                copy_tick[c] = t
            else:
                tanh_tick[(c, part)] = t

        def chunk_slice(view, c):
            return view[:, :, v0s[c]:v0s[c] + VS[c]]

        def bcast(ap, c):
            return ap[:, :, None, :].to_broadcast((128, VS[c], II, D))

        # --- engine programs ----------------------------------------------
        @block.sync
        def _(sync):
            for c in range(nchunk):
                sync.dma_start(
                    xts[c][:], chunk_slice(xw, c)
                ).then_inc(in_sems[c], 16)
            # Output DMAs issue from SP so the ACT queue stays free for the
            # tanh stream.
            h2 = II // 2
            for c in range(nchunk):
                if c in SPLIT_OUT:
                    sync.wait_ge(act_sem, tanh_tick[(c, 0)])
                    sync.dma_start(
                        chunk_slice(yw, c)[:, :, :, 0:h2],
                        ots[c][:, :, 0:h2],
                    ).then_inc(out_sems[c], 16)
                    sync.wait_ge(act_sem, tanh_tick[(c, 1)])
                    sync.dma_start(
                        chunk_slice(yw, c)[:, :, :, h2:II],
                        ots[c][:, :, h2:II],
                    ).then_inc(out_sems[c], 16)
                else:
                    sync.wait_ge(act_sem, tanh_tick[(c, 0)])
                    sync.dma_start(
                        chunk_slice(yw, c), ots[c][:]
                    ).then_inc(out_sems[c], 16)
            for c in range(nchunk):
                sync.wait_ge(out_sems[c], 16 * (2 if c in SPLIT_OUT else 1))

        @block.vector
        def _(vector):
            # Finish the selector: (c >> log2(S)) == 0 as f32.  (These two
            # scalar ops are not supported by walrus on the Pool engine.)
            vector.wait_ge(sel_sem, 1)
            vector.tensor_scalar(
                out=sel_i[:], in0=sel_i[:], scalar1=S.bit_length() - 1,
                scalar2=None, op0=mybir.AluOpType.logical_shift_right,
            ).then_inc(sel_sem)
            vector.wait_ge(sel_sem, 2)
            vector.tensor_scalar(
                out=sel_t[:], in0=sel_i[:], scalar1=0, scalar2=None,
                op0=mybir.AluOpType.is_equal,
            ).then_inc(sel_sem)
            for kind, c in dve_ops:
                if kind == "reduce":
                    vector.wait_ge(in_sems[c], 16)
                    vector.tensor_reduce(
                        out=partials[c][:],
                        in_=xts[c][:].rearrange("p v i d -> p v d i"),
                        axis=mybir.AxisListType.X,
                        op=mybir.AluOpType.add,
                    ).then_inc(dve_sem)
                elif kind == "hreduce":
                    vector.wait_ge(pool_sem, half_tick[c])
                    vector.tensor_reduce(
                        out=partials[c][:],
                        in_=tr4[c][:].rearrange("p v i d -> p v d i"),
                        axis=mybir.AxisListType.X,
                        op=mybir.AluOpType.add,
                    ).then_inc(dve_sem)
                else:
                    vector.wait_ge(pe_sem, c + 1)
                    vector.tensor_mul(
                        ots[c][:], xts[c][:], bcast(sums[c][:], c)
                    ).then_inc(dve_sem)

        @block.tensor
        def _(tensor):
            tensor.wait_ge(sel_sem, 3)
            for c in range(nchunk):
                if CLS[c] == "t":
                    tensor.wait_ge(pool_sem, tree_tick[c])
                else:
                    tensor.wait_ge(dve_sem, reduce_tick[c])
                if c >= nbank:
                    prev = c - nbank
                    if CLS[prev] == "m":
                        tensor.wait_ge(pool_sem, mult_tick[prev])
                    else:
                        tensor.wait_ge(dve_sem, mult_tick[prev])
                tensor.matmul(
                    sums[c][:], sel_t[:],
                    partials[c][:].rearrange("p v d -> p (v d)"),
                    start=True, stop=True,
                ).then_inc(pe_sem)

        @block.gpsimd
        def _(gpsimd):
            # Build the selector on-chip (no DMA).  With c[p, m] =
            # p - S*(m//S)  (iota: free pattern [-S over m//S, 0 over m%S],
            # channel_multiplier 1), p//S == m//S iff 0 <= c < S, i.e.
            # (c >> log2(S)) == 0 after a LOGICAL shift (negatives wrap to
            # huge positives).  Same-queue chaining needs sem waits.
            gpsimd.iota(
                sel_i[:].rearrange("p (a b) -> p a b", b=S),
                pattern=[[-S, NB0], [0, S]],
                base=0, channel_multiplier=1,
            ).then_inc(sel_sem)
            h = II // 2
            ptick = 0
            for kind, c in pool_ops:
                # Engine pipelines are deep: a dependent op on the SAME queue
                # still needs a semaphore wait on its producer.
                if kind == "add4":
                    gpsimd.wait_ge(in_sems[c], 16)
                    gpsimd.tensor_add(
                        tr4[c][:], xts[c][:, :, 0:h], xts[c][:, :, h:II]
                    ).then_inc(pool_sem)
                elif kind == "add2":
                    gpsimd.wait_ge(pool_sem, ptick)
                    gpsimd.tensor_add(
                        tr2[c][:], tr4[c][:, :, 0:h // 2], tr4[c][:, :, h // 2:h]
                    ).then_inc(pool_sem)
                elif kind == "add1":
                    gpsimd.wait_ge(pool_sem, ptick)
                    gpsimd.tensor_add(
                        partials[c][:, :, None, :],
                        tr2[c][:, :, 0:1], tr2[c][:, :, 1:2],
                    ).then_inc(pool_sem)
                else:  # 'm' multiply
                    # xts[c] is transitively ready: copy_c <- mm_c <-
                    # reduce_c <- in-DMA c.
                    gpsimd.wait_ge(act_sem, copy_tick[c])
                    gpsimd.tensor_mul(
                        ots[c][:], xts[c][:], bcast(sums_sb[c][:], c)
                    ).then_inc(pool_sem)
                ptick += 1

        @block.scalar
        def _(scalar):
            h2 = II // 2
            for kind, c, part in act_ops:
                if kind == "copy":
                    scalar.wait_ge(pe_sem, c + 1)
                    # sums_sb = sums / I, so 'm'-chunk tanh uses scale=1.
                    scalar.mul(
                        sums_sb[c][:], sums[c][:], 1.0 / I
                    ).then_inc(act_sem)
                    continue
                if part == 0:  # the multiply covers both halves
                    if CLS[c] == "m":
                        scalar.wait_ge(pool_sem, mult_tick[c])
                    else:
                        scalar.wait_ge(dve_sem, mult_tick[c])
                scale = 1.0 if CLS[c] == "m" else 1.0 / I
                tgt = (ots[c][:, :, part * h2:(part + 1) * h2]
                       if c in SPLIT_OUT else ots[c][:])
                scalar.activation(
                    out=tgt, in_=tgt,
                    func=mybir.ActivationFunctionType.Tanh, scale=scale,
                ).then_inc(act_sem)

    return nc


def _prepare():
    """Build the Bass module once and cache a jitted shard_map executable
    (mirrors concourse.bass2jax.run_bass_via_pjrt, which re-traces on every
    call)."""
    import jax
    from jax.experimental.shard_map import shard_map
    from jax.sharding import Mesh, PartitionSpec
    from concourse import bass2jax, mybir as mb

    nc = _build()
    bass2jax.install_neuronx_cc_hook()
    assert nc.dbg_addr is None

    partition_name = (nc.partition_id_tensor.name
                      if nc.partition_id_tensor else None)
    in_names, out_names, out_avals = [], [], []
    for alloc in nc.m.functions[0].allocations:
        if not isinstance(alloc, mb.MemoryLocationSet):
            continue
        name = alloc.memorylocations[0].name
        if alloc.kind == "ExternalInput":
            if name != partition_name:
                in_names.append(name)
        elif alloc.kind == "ExternalOutput":
            shape = tuple(alloc.tensor_shape)
            out_avals.append(
                jax.core.ShapedArray(shape, mb.dt.np(alloc.dtype)))
            out_names.append(name)
    n_params = len(in_names)
    all_in_names = in_names + out_names
    if partition_name is not None:
        all_in_names = all_in_names + [partition_name]
    donate = tuple(range(n_params, n_params + len(out_names)))

    def _body(*args):
        operands = list(args)
        if partition_name is not None:
            operands.append(bass2jax.partition_id_tensor())
        return tuple(bass2jax._bass_exec_p.bind(
            *operands,
            out_avals=tuple(out_avals),
            in_names=tuple(all_in_names),
            out_names=tuple(out_names),
            lowering_input_output_aliases=(),
            sim_require_finite=True,
            sim_require_nnan=True,
            nc=nc,
        ))

    devices = jax.devices()[:N_CORES]
    assert len(devices) == N_CORES, devices
    mesh = Mesh(np.asarray(devices), ("core",))
    nio = n_params + len(out_names)
    sharded = jax.jit(
        shard_map(_body, mesh=mesh,
                  in_specs=(PartitionSpec("core"),) * nio,
                  out_specs=(PartitionSpec("core"),) * len(out_names),
                  check_rep=False),
        donate_argnums=donate, keep_unused=True,
    )
    _cache.update(nc=nc, sharded=sharded, in_names=in_names,
                  out_names=out_names, out_avals=out_avals)


def _run(emb: np.ndarray, trace: bool = False):
    if trace:  # profiling path via bass_utils (no cached jit)
        nc = _cache.get("nc") or _build()
        in_maps = [
            {"x": np.ascontiguousarray(emb[c * BPC:(c + 1) * BPC])}
            for c in range(N_CORES)
        ]
        res = run_bass_kernel_spmd(nc, in_maps, list(range(N_CORES)),
                                   trace=True)
        return np.concatenate([r["y"] for r in res.results], axis=0), res

    if "sharded" not in _cache:
        _prepare()
    assert _cache["in_names"] == ["x"] and _cache["out_names"] == ["y"]
    zeros = [np.zeros((N_CORES * a.shape[0], *a.shape[1:]), a.dtype)
             for a in _cache["out_avals"]]
    out_arrs = _cache["sharded"](np.ascontiguousarray(emb), *zeros)
    return np.asarray(out_arrs[0]), None


def kernel(embeddings: np.ndarray) -> np.ndarray:
    emb = np.asarray(embeddings, dtype=np.float32)
    try:
        out, _ = _run(emb, trace=False)
    except Exception:
        # Fallback: stock per-call path (same NEFF, re-traced each call).
        nc = _cache.get("nc") or _build()
        in_maps = [
            {"x": np.ascontiguousarray(emb[c * BPC:(c + 1) * BPC])}
            for c in range(N_CORES)
        ]
        res = run_bass_kernel_spmd(nc, in_maps, list(range(N_CORES)))
        out = np.concatenate([r["y"] for r in res.results], axis=0)
    return out
